# revision 23
# baseline (speedup 1.0000x reference)
"""Trainium2 Bass kernel for nn_IntraAgg (GNN mean-neighbor aggregation).

reference:
    valid[b,k] = k < neigh_counts[b]
    out = relu( (sum_k valid[b,k] * features[neigh_idx[b,k]]) / neigh_counts[b] )

Strategy (8 NeuronCores, data-parallel over the batch):
  - shard neigh_idx/neigh_counts along B (8192 -> 1024 per core), replicate
    the features table.
  - The gather is done with the GPSIMD `dma_gather` custom instruction
    (one instruction moves thousands of 256B rows; amortizes the ~1us
    SWDGE fixed cost that dominates per-slot indirect DMAs).  dma_gather
    indices are int16, so one instruction can only address a 32768-row
    window of the 1M-row table.  Hence two phases:

    Phase 1: per core, gather the core's ~17k unique needed rows, sorted
      by table index, with one dma_gather per 32K window (31 windows),
      then write each window's rows back to an HBM staging buffer with a
      plain (affine) HWDGE DMA.  Staging has < 32K rows, so it is fully
      int16-addressable.
    Phase 2: one dma_gather per 128-node block pulls the block's
      neighbor rows from staging in slot order [128, kj, 64]; a strided
      DVE reduce sums over the kj neighbor slots; ACT applies
      relu(x * (1/count)); result is DMAed out.

  - Nodes are count-sorted per core so block b only needs k_sched[b]
    neighbor slots.  Invalid (k >= count) slots point at a zeroed
    staging row, so no per-block zero-fill or bounds check is needed.
"""

import numpy as np

N_NODES = 1_000_000
FEAT_DIM = 64
BATCH = 8192
MAX_NEIGH = 32
N_CORES = 8
BLK = 128    # nodes per block (SBUF partition dim)
WIN = 32768  # dma_gather int16 index window (table rows per instruction)

_KERNEL_CACHE = {}
_LAST_SCHED = None  # set by prep_core_inputs; consumed by kernel()


def _split_multi_waits(nc):
    """walrus codegen accepts at most one sync-wait per instruction: hoist
    extra waits onto NoOp instructions inserted just before."""
    import bass_rust

    for fn in nc.m.functions:
        for bb in fn.blocks:
            new_list = []
            for inst in bb.instructions:
                si = inst.sync_info
                if si is not None and si.on_wait is not None and len(si.on_wait) > 1:
                    waits = list(si.on_wait)
                    for j, w in enumerate(waits[:-1]):
                        nop = bass_rust.InstNoOp(name=f"{inst.name}-sw{j}")
                        nop.engine = inst.engine
                        nop.sync_info = bass_rust.SyncInfo(on_wait=[w], on_update=[])
                        new_list.append(nop)
                    inst.sync_info = bass_rust.SyncInfo(
                        on_wait=[waits[-1]], on_update=list(si.on_update or [])
                    )
                new_list.append(inst)
            bb.instructions = new_list


def build_nc(n_nodes=N_NODES, b_loc=BATCH // N_CORES, k=MAX_NEIGH, d=FEAT_DIM,
             legalize=True, k_sched=None, cw_sched=None):
    """Build the per-core Bass program (SPMD: same program on every core).

    cw_sched[w] = number of 128-row staging blocks for window w (shared
    across cores; 0 = window empty, no instruction emitted)."""
    from concourse import bass, mybir, library_config
    from concourse.tile import TileContext

    assert b_loc % BLK == 0
    nblk = b_loc // BLK
    assert k_sched is not None and cw_sched is not None
    assert len(k_sched) == nblk and all(1 <= kj <= k for kj in k_sched)

    nwin = len(cw_sched)
    c_max = max(cw_sched)
    ns = 128 * sum(cw_sched)       # staging rows used by gathered data
    zp = ns                        # zero-row block position
    ns_tot = ns + 128
    assert ns_tot <= 32767, ns_tot
    s1_cols = sum(8 * c for c in cw_sched)    # phase-1 idx cols (128*c/16)
    s2_cols = sum(8 * kj for kj in k_sched)   # phase-2 idx cols

    nc = bass.Bass(num_swdge_queues=4)
    feat = nc.declare_dram_parameter("feat", [n_nodes, d], mybir.dt.float32,
                                     isOutput=False)
    gidx = nc.declare_dram_parameter("gidx", [128, s1_cols], mybir.dt.int16,
                                     isOutput=False)
    sidx = nc.declare_dram_parameter("sidx", [128, s2_cols], mybir.dt.int16,
                                     isOutput=False)
    recip = nc.declare_dram_parameter("recip", [b_loc, 1], mybir.dt.float32,
                                      isOutput=False)
    out = nc.declare_dram_parameter("out", [b_loc, d], mybir.dt.float32,
                                    isOutput=True)

    fp32 = mybir.dt.float32
    with TileContext(nc) as tc:
        with tc.tile_pool(name="const", bufs=1) as constp, \
             tc.tile_pool(name="stgp", bufs=1, space="DRAM") as stgp, \
             tc.tile_pool(name="p1p", bufs=8) as p1p, \
             tc.tile_pool(name="recp", bufs=4) as recp, \
             tc.tile_pool(name="gp", bufs=4) as gp, \
             tc.tile_pool(name="redp", bufs=3) as redp, \
             tc.tile_pool(name="outp", bufs=3) as outp:
            nc.gpsimd.load_library(library_config.mlp)

            # one shared register per distinct idx-count (to_reg per call
            # would exhaust the register file)
            nreg = {}
            for cw in set(c for c in cw_sched if c):
                nreg[128 * cw] = nc.gpsimd.to_reg(128 * cw)
            for kj in k_sched:
                for k0 in range(0, kj, 16):
                    km = min(16, kj - k0)
                    if 128 * km not in nreg:
                        nreg[128 * km] = nc.gpsimd.to_reg(128 * km)

            gt = constp.tile([128, s1_cols], mybir.dt.int16)
            nc.sync.dma_start(out=gt[:], in_=gidx[:, :])
            st = constp.tile([128, s2_cols], mybir.dt.int16)
            nc.sync.dma_start(out=st[:], in_=sidx[:, :])

            staging = stgp.tile([ns_tot, d], fp32)
            # zero block for invalid (k >= count) slots
            zt = constp.tile([128, d], fp32)
            nc.vector.memset(zt[:], 0.0)
            nc.sync.dma_start(
                out=staging[zp:zp + 128, :].rearrange("(c p) d -> p c d", p=128),
                in_=zt[:].rearrange("p (c d) -> p c d", d=d),
            )

            # ---- phase 1: windowed gathers from the table into staging ----
            # Round-robin the 4 SWDGE queues: each queue is serviced by its
            # own GPSIMD Q7 core pair, so descriptor generation (the
            # bottleneck, ~8ns/row on one pair) runs up to 4-way parallel.
            qn = 0
            off = 0   # staging row offset
            col = 0   # gidx col offset
            for w in range(nwin):
                cw = cw_sched[w]
                if cw == 0:
                    continue
                n_idx = 128 * cw
                wlo = w * WIN
                whi = min(wlo + WIN, n_nodes)
                t1 = p1p.tile([128, c_max * d], fp32, tag="t1")
                t1v = t1[:, :cw * d].rearrange("p (c d) -> p c d", d=d)
                nc.gpsimd.dma_gather(
                    t1v,
                    feat[wlo:whi, :],
                    gt[:, col:col + 8 * cw],
                    n_idx,
                    nreg[n_idx],
                    d,
                    # >64 descriptors per SDMA engine don't fit one packet
                    single_packet=(n_idx <= 1024),
                    queue_num=qn,
                )
                qn = (qn + 1) % 4
                nc.sync.dma_start(
                    out=staging[off:off + n_idx, :].rearrange(
                        "(c p) d -> p c d", p=128),
                    in_=t1v,
                )
                off += n_idx
                col += 8 * cw

            # ---- phase 2: per-block slot gathers from staging + reduce ----
            col2 = 0
            for b in range(nblk):
                kj = k_sched[b]
                sl = slice(b * BLK, (b + 1) * BLK)
                rt = recp.tile([BLK, 1], fp32)
                nc.sync.dma_start(out=rt[:], in_=recip[sl, :])

                g = gp.tile([BLK, k * d], fp32, tag="g")
                # split the block's slot gather into <=8-slot (1024-idx)
                # chunks round-robined over the 4 SWDGE queues
                for k0 in range(0, kj, 16):
                    km = min(16, kj - k0)
                    nc.gpsimd.dma_gather(
                        g[:, k0 * d:(k0 + km) * d].rearrange(
                            "p (k d) -> p k d", d=d),
                        staging[:, :],
                        st[:, col2 + 8 * k0:col2 + 8 * (k0 + km)],
                        128 * km,
                        nreg[128 * km],
                        d,
                        single_packet=(128 * km <= 1024),
                        queue_num=qn,
                    )
                    qn = (qn + 1) % 4
                col2 += 8 * kj

                red = redp.tile([BLK, d], fp32)
                nc.vector.tensor_reduce(
                    out=red[:],
                    in_=g[:, :kj * d].rearrange("p (k d) -> p d k", d=d),
                    axis=mybir.AxisListType.X,
                    op=mybir.AluOpType.add,
                )
                o = outp.tile([BLK, d], fp32)
                nc.scalar.activation(
                    out=o[:],
                    in_=red[:],
                    func=mybir.ActivationFunctionType.Relu,
                    scale=rt[:, :1],
                )
                nc.sync.dma_start(out=out[sl, :], in_=o[:])

    if legalize:
        _split_multi_waits(nc)
    # raw Bass skips Bacc's codegen pass for extended/pseudo instructions
    # (dma_gather, load_library); without it walrus sees empty .instr bytes
    # and fails with "ISA wrong length".
    mybir.codegen_inst_isa_subclasses(nc)
    return nc


def _wrap16(vals, cols, fill):
    """Pack `vals` into the [128, cols] int16 wrapped layout dma_gather
    expects: position i at [i%16, i//16], replicated across the 8
    16-partition groups."""
    flat = np.full(16 * cols, fill, dtype=np.int16)
    flat[:len(vals)] = vals
    arr = flat.reshape(cols, 16).T  # position i = col*16 + p at [p, col]
    return np.tile(arr, (8, 1))


def prep_core_inputs(features, neigh_idx, neigh_counts, n_cores=N_CORES):
    """Host-side sharding + index-space preprocessing (no feature data is
    touched on the host).  Returns (in_maps, orders, k_sched) and stores the
    shared phase-1 window schedule in _LAST_SCHED."""
    global _LAST_SCHED
    n_nodes = features.shape[0]
    b = neigh_idx.shape[0]
    b_loc = b // n_cores
    k = neigh_idx.shape[1]
    nblk = b_loc // BLK
    nwin = (n_nodes + WIN - 1) // WIN

    idx_all = np.asarray(neigh_idx, dtype=np.int64)
    counts = np.asarray(neigh_counts, dtype=np.int64)
    recip = (1.0 / counts.astype(np.float64)).astype(np.float32)[:, None]
    feat = np.ascontiguousarray(np.asarray(features, dtype=np.float32))

    # per-core count-sort and unique-row analysis
    cores = []
    k_sched = np.ones(nblk, dtype=np.int64)
    nw_max = np.zeros(nwin, dtype=np.int64)
    for c in range(n_cores):
        sl = slice(c * b_loc, (c + 1) * b_loc)
        cnt_c = counts[sl]
        order = np.argsort(-cnt_c, kind="stable")
        sorted_cnt = cnt_c[order]
        k_sched = np.maximum(
            k_sched, sorted_cnt.reshape(nblk, BLK).max(axis=1))
        idx_c = idx_all[sl][order]              # [b_loc, k] count-sorted
        valid = (np.arange(k)[None, :] < sorted_cnt[:, None])
        uniq = np.unique(idx_c[valid])          # sorted unique table rows
        win_of = uniq // WIN
        n_w = np.bincount(win_of, minlength=nwin)
        nw_max = np.maximum(nw_max, n_w)
        cores.append((order, sorted_cnt, idx_c, valid, uniq, n_w))

    cw_sched = tuple(int(-(-n // BLK)) if n > 0 else 0 for n in nw_max)
    ns = 128 * sum(cw_sched)
    zp = ns
    k_sched = tuple(int(x) for x in k_sched)
    _LAST_SCHED = cw_sched

    in_maps, orders = [], []
    for c in range(n_cores):
        order, sorted_cnt, idx_c, valid, uniq, n_w = cores[c]
        # staging position of each unique row (window-major, padded blocks)
        pos_uniq = np.empty(len(uniq), dtype=np.int64)
        off = 0
        gidx_cols = []
        for w in range(nwin):
            cw = cw_sched[w]
            if cw == 0:
                continue
            lo = np.searchsorted(uniq, w * WIN)
            hi = np.searchsorted(uniq, (w + 1) * WIN)
            nwc = hi - lo
            pos_uniq[lo:hi] = off + np.arange(nwc)
            local = (uniq[lo:hi] - w * WIN).astype(np.int16)
            fill = local[0] if nwc > 0 else np.int16(0)
            gidx_cols.append(_wrap16(local, 8 * cw, fill))
            off += 128 * cw
        gidx16 = np.concatenate(gidx_cols, axis=1)

        # phase-2 slot indices: block-major, position i = k*128 + p
        sidx_cols = []
        for bb in range(nblk):
            kj = k_sched[bb]
            blk_idx = idx_c[bb * BLK:(bb + 1) * BLK, :kj]        # [128, kj]
            blk_valid = valid[bb * BLK:(bb + 1) * BLK, :kj]
            pos = np.full((BLK, kj), zp, dtype=np.int64)
            pos[blk_valid] = pos_uniq[
                np.searchsorted(uniq, blk_idx[blk_valid])]
            # position i = k*128 + p  ->  stream k-major
            stream = pos.T.reshape(-1)                           # [kj*128]
            sidx_cols.append(_wrap16(stream.astype(np.int16), 8 * kj, zp))
        sidx16 = np.concatenate(sidx_cols, axis=1)

        orders.append(order)
        in_maps.append({
            "feat": feat,
            "gidx": np.ascontiguousarray(gidx16),
            "sidx": np.ascontiguousarray(sidx16),
            "recip": np.ascontiguousarray(recip[c * b_loc:(c + 1) * b_loc][order]),
        })
    return in_maps, orders, k_sched


def kernel(features, neigh_idx, neigh_counts):
    from concourse.bass_utils import run_bass_kernel_spmd

    in_maps, orders, k_sched = prep_core_inputs(
        features, neigh_idx, neigh_counts)
    cw_sched = _LAST_SCHED
    key = ("nc", N_NODES, BATCH // N_CORES, MAX_NEIGH, FEAT_DIM, k_sched)
    if key not in _KERNEL_CACHE:
        _KERNEL_CACHE[key] = build_nc(k_sched=list(k_sched),
                                      cw_sched=list(cw_sched))
    nc = _KERNEL_CACHE[key]

    res = run_bass_kernel_spmd(nc, in_maps, list(range(N_CORES)))
    b_loc = BATCH // N_CORES
    out = np.empty((BATCH, FEAT_DIM), dtype=np.float32)
    for c in range(N_CORES):
        out_c = np.empty((b_loc, FEAT_DIM), dtype=np.float32)
        out_c[orders[c]] = res.results[c]["out"]
        out[c * b_loc:(c + 1) * b_loc] = out_c
    return out


# revision 25
# speedup vs baseline: 1.1318x; 1.1318x over previous
"""Trainium2 Bass kernel for nn_IntraAgg (GNN mean-neighbor aggregation).

reference:
    valid[b,k] = k < neigh_counts[b]
    out = relu( (sum_k valid[b,k] * features[neigh_idx[b,k]]) / neigh_counts[b] )

Strategy (8 NeuronCores, data-parallel over the batch):
  - shard neigh_idx/neigh_counts along B (8192 -> 1024 per core), replicate
    the features table.
  - The gather is done with the GPSIMD `dma_gather` custom instruction
    (one instruction moves thousands of 256B rows; amortizes the ~1us
    SWDGE fixed cost that dominates per-slot indirect DMAs).  dma_gather
    indices are int16, so one instruction can only address a 32768-row
    window of the 1M-row table.  Hence two phases:

    Phase 1: per core, gather the core's ~17k unique needed rows, sorted
      by table index, with one dma_gather per 32K window (31 windows),
      then write each window's rows back to an HBM staging buffer with a
      plain (affine) HWDGE DMA.  Staging has < 32K rows, so it is fully
      int16-addressable.
    Phase 2: one dma_gather per 128-node block pulls the block's
      neighbor rows from staging in slot order [128, kj, 64]; a strided
      DVE reduce sums over the kj neighbor slots; ACT applies
      relu(x * (1/count)); result is DMAed out.

  - Nodes are count-sorted per core so block b only needs k_sched[b]
    neighbor slots.  Invalid (k >= count) slots point at a zeroed
    staging row, so no per-block zero-fill or bounds check is needed.
"""

import numpy as np

N_NODES = 1_000_000
FEAT_DIM = 64
BATCH = 8192
MAX_NEIGH = 32
N_CORES = 8
BLK = 128    # nodes per block (SBUF partition dim)
WIN = 32768  # dma_gather int16 index window (table rows per instruction)

_KERNEL_CACHE = {}
_LAST_SCHED = None  # set by prep_core_inputs; consumed by kernel()


def _split_multi_waits(nc):
    """walrus codegen accepts at most one sync-wait per instruction: hoist
    extra waits onto NoOp instructions inserted just before."""
    import bass_rust

    for fn in nc.m.functions:
        for bb in fn.blocks:
            new_list = []
            for inst in bb.instructions:
                si = inst.sync_info
                if si is not None and si.on_wait is not None and len(si.on_wait) > 1:
                    waits = list(si.on_wait)
                    for j, w in enumerate(waits[:-1]):
                        nop = bass_rust.InstNoOp(name=f"{inst.name}-sw{j}")
                        nop.engine = inst.engine
                        nop.sync_info = bass_rust.SyncInfo(on_wait=[w], on_update=[])
                        new_list.append(nop)
                    inst.sync_info = bass_rust.SyncInfo(
                        on_wait=[waits[-1]], on_update=list(si.on_update or [])
                    )
                new_list.append(inst)
            bb.instructions = new_list


def build_nc(n_nodes=N_NODES, b_loc=BATCH // N_CORES, k=MAX_NEIGH, d=FEAT_DIM,
             legalize=True, k_sched=None, cw_sched=None):
    """Build the per-core Bass program (SPMD: same program on every core).

    cw_sched[w] = number of 128-row staging blocks for window w (shared
    across cores; 0 = window empty, no instruction emitted)."""
    from concourse import bass, mybir, library_config
    from concourse.tile import TileContext

    assert b_loc % BLK == 0
    nblk = b_loc // BLK
    assert k_sched is not None and cw_sched is not None
    assert len(k_sched) == nblk and all(1 <= kj <= k for kj in k_sched)

    nwin = len(cw_sched)
    c_max = max(cw_sched)
    ns = 128 * sum(cw_sched)       # staging rows used by gathered data
    zp = ns                        # zero-row block position
    ns_tot = ns + 128
    assert ns_tot <= 32767, ns_tot
    s1_cols = sum(8 * c for c in cw_sched)    # phase-1 idx cols (128*c/16)
    s2_cols = sum(8 * kj for kj in k_sched)   # phase-2 idx cols

    nc = bass.Bass(num_swdge_queues=4)
    feat = nc.declare_dram_parameter("feat", [n_nodes, d], mybir.dt.float32,
                                     isOutput=False)
    gidx = nc.declare_dram_parameter("gidx", [128, s1_cols], mybir.dt.int16,
                                     isOutput=False)
    sidx = nc.declare_dram_parameter("sidx", [128, s2_cols], mybir.dt.int16,
                                     isOutput=False)
    recip = nc.declare_dram_parameter("recip", [b_loc, 1], mybir.dt.float32,
                                      isOutput=False)
    out = nc.declare_dram_parameter("out", [b_loc, d], mybir.dt.float32,
                                    isOutput=True)

    fp32 = mybir.dt.float32
    with TileContext(nc) as tc:
        with tc.tile_pool(name="const", bufs=1) as constp, \
             tc.tile_pool(name="stgp", bufs=1, space="DRAM") as stgp, \
             tc.tile_pool(name="p1p", bufs=12) as p1p, \
             tc.tile_pool(name="recp", bufs=4) as recp, \
             tc.tile_pool(name="gp", bufs=4) as gp, \
             tc.tile_pool(name="redp", bufs=3) as redp, \
             tc.tile_pool(name="outp", bufs=3) as outp:
            nc.gpsimd.load_library(library_config.mlp)

            # one shared register per distinct idx-count (to_reg per call
            # would exhaust the register file)
            nreg = {}
            for cw in set(c for c in cw_sched if c):
                nreg[128 * cw] = nc.gpsimd.to_reg(128 * cw)
            for kj in k_sched:
                for k0 in range(0, kj, 8):
                    km = min(8, kj - k0)
                    if 128 * km not in nreg:
                        nreg[128 * km] = nc.gpsimd.to_reg(128 * km)

            gt = constp.tile([128, s1_cols], mybir.dt.int16)
            nc.sync.dma_start(out=gt[:], in_=gidx[:, :])
            st = constp.tile([128, s2_cols], mybir.dt.int16)
            nc.sync.dma_start(out=st[:], in_=sidx[:, :])

            staging = stgp.tile([ns_tot, d], fp32)
            # zero block for invalid (k >= count) slots
            zt = constp.tile([128, d], fp32)
            nc.vector.memset(zt[:], 0.0)
            nc.sync.dma_start(
                out=staging[zp:zp + 128, :].rearrange("(c p) d -> p c d", p=128),
                in_=zt[:].rearrange("p (c d) -> p c d", d=d),
            )

            # ---- phase 1: windowed gathers from the table into staging ----
            # Round-robin the 4 SWDGE queues: each queue is serviced by its
            # own GPSIMD Q7 core pair, so descriptor generation (the
            # bottleneck, ~8ns/row on one pair) runs up to 4-way parallel.
            qn = 0
            off = 0   # staging row offset
            col = 0   # gidx col offset
            for w in range(nwin):
                cw = cw_sched[w]
                if cw == 0:
                    continue
                n_idx = 128 * cw
                wlo = w * WIN
                whi = min(wlo + WIN, n_nodes)
                t1 = p1p.tile([128, c_max * d], fp32, tag="t1")
                t1v = t1[:, :cw * d].rearrange("p (c d) -> p c d", d=d)
                nc.gpsimd.dma_gather(
                    t1v,
                    feat[wlo:whi, :],
                    gt[:, col:col + 8 * cw],
                    n_idx,
                    nreg[n_idx],
                    d,
                    # >64 descriptors per SDMA engine don't fit one packet
                    single_packet=(n_idx <= 1024),
                    queue_num=qn,
                )
                qn = (qn + 1) % 4
                nc.sync.dma_start(
                    out=staging[off:off + n_idx, :].rearrange(
                        "(c p) d -> p c d", p=128),
                    in_=t1v,
                )
                off += n_idx
                col += 8 * cw

            # ---- phase 2: per-block slot gathers from staging + reduce ----
            col2 = 0
            for b in range(nblk):
                kj = k_sched[b]
                sl = slice(b * BLK, (b + 1) * BLK)
                rt = recp.tile([BLK, 1], fp32)
                nc.sync.dma_start(out=rt[:], in_=recip[sl, :])

                g = gp.tile([BLK, k * d], fp32, tag="g")
                # split the block's slot gather into <=8-slot (1024-idx)
                # chunks round-robined over the 4 SWDGE queues
                for k0 in range(0, kj, 8):
                    km = min(8, kj - k0)
                    nc.gpsimd.dma_gather(
                        g[:, k0 * d:(k0 + km) * d].rearrange(
                            "p (k d) -> p k d", d=d),
                        staging[:, :],
                        st[:, col2 + 8 * k0:col2 + 8 * (k0 + km)],
                        128 * km,
                        nreg[128 * km],
                        d,
                        queue_num=qn,
                    )
                    qn = (qn + 1) % 4
                col2 += 8 * kj

                red = redp.tile([BLK, d], fp32)
                nc.vector.tensor_reduce(
                    out=red[:],
                    in_=g[:, :kj * d].rearrange("p (k d) -> p d k", d=d),
                    axis=mybir.AxisListType.X,
                    op=mybir.AluOpType.add,
                )
                o = outp.tile([BLK, d], fp32)
                nc.scalar.activation(
                    out=o[:],
                    in_=red[:],
                    func=mybir.ActivationFunctionType.Relu,
                    scale=rt[:, :1],
                )
                nc.sync.dma_start(out=out[sl, :], in_=o[:])

    if legalize:
        _split_multi_waits(nc)
    # raw Bass skips Bacc's codegen pass for extended/pseudo instructions
    # (dma_gather, load_library); without it walrus sees empty .instr bytes
    # and fails with "ISA wrong length".
    mybir.codegen_inst_isa_subclasses(nc)
    return nc


def _wrap16(vals, cols, fill):
    """Pack `vals` into the [128, cols] int16 wrapped layout dma_gather
    expects: position i at [i%16, i//16], replicated across the 8
    16-partition groups."""
    flat = np.full(16 * cols, fill, dtype=np.int16)
    flat[:len(vals)] = vals
    arr = flat.reshape(cols, 16).T  # position i = col*16 + p at [p, col]
    return np.tile(arr, (8, 1))


def prep_core_inputs(features, neigh_idx, neigh_counts, n_cores=N_CORES):
    """Host-side sharding + index-space preprocessing (no feature data is
    touched on the host).  Returns (in_maps, orders, k_sched) and stores the
    shared phase-1 window schedule in _LAST_SCHED."""
    global _LAST_SCHED
    n_nodes = features.shape[0]
    b = neigh_idx.shape[0]
    b_loc = b // n_cores
    k = neigh_idx.shape[1]
    nblk = b_loc // BLK
    nwin = (n_nodes + WIN - 1) // WIN

    idx_all = np.asarray(neigh_idx, dtype=np.int64)
    counts = np.asarray(neigh_counts, dtype=np.int64)
    recip = (1.0 / counts.astype(np.float64)).astype(np.float32)[:, None]
    feat = np.ascontiguousarray(np.asarray(features, dtype=np.float32))

    # per-core count-sort and unique-row analysis
    cores = []
    k_sched = np.ones(nblk, dtype=np.int64)
    nw_max = np.zeros(nwin, dtype=np.int64)
    for c in range(n_cores):
        sl = slice(c * b_loc, (c + 1) * b_loc)
        cnt_c = counts[sl]
        order = np.argsort(-cnt_c, kind="stable")
        sorted_cnt = cnt_c[order]
        k_sched = np.maximum(
            k_sched, sorted_cnt.reshape(nblk, BLK).max(axis=1))
        idx_c = idx_all[sl][order]              # [b_loc, k] count-sorted
        valid = (np.arange(k)[None, :] < sorted_cnt[:, None])
        uniq = np.unique(idx_c[valid])          # sorted unique table rows
        win_of = uniq // WIN
        n_w = np.bincount(win_of, minlength=nwin)
        nw_max = np.maximum(nw_max, n_w)
        cores.append((order, sorted_cnt, idx_c, valid, uniq, n_w))

    cw_sched = tuple(int(-(-n // BLK)) if n > 0 else 0 for n in nw_max)
    ns = 128 * sum(cw_sched)
    zp = ns
    k_sched = tuple(int(x) for x in k_sched)
    _LAST_SCHED = cw_sched

    in_maps, orders = [], []
    for c in range(n_cores):
        order, sorted_cnt, idx_c, valid, uniq, n_w = cores[c]
        # staging position of each unique row (window-major, padded blocks)
        pos_uniq = np.empty(len(uniq), dtype=np.int64)
        off = 0
        gidx_cols = []
        for w in range(nwin):
            cw = cw_sched[w]
            if cw == 0:
                continue
            lo = np.searchsorted(uniq, w * WIN)
            hi = np.searchsorted(uniq, (w + 1) * WIN)
            nwc = hi - lo
            pos_uniq[lo:hi] = off + np.arange(nwc)
            local = (uniq[lo:hi] - w * WIN).astype(np.int16)
            fill = local[0] if nwc > 0 else np.int16(0)
            gidx_cols.append(_wrap16(local, 8 * cw, fill))
            off += 128 * cw
        gidx16 = np.concatenate(gidx_cols, axis=1)

        # phase-2 slot indices: block-major, position i = k*128 + p
        sidx_cols = []
        for bb in range(nblk):
            kj = k_sched[bb]
            blk_idx = idx_c[bb * BLK:(bb + 1) * BLK, :kj]        # [128, kj]
            blk_valid = valid[bb * BLK:(bb + 1) * BLK, :kj]
            pos = np.full((BLK, kj), zp, dtype=np.int64)
            pos[blk_valid] = pos_uniq[
                np.searchsorted(uniq, blk_idx[blk_valid])]
            # position i = k*128 + p  ->  stream k-major
            stream = pos.T.reshape(-1)                           # [kj*128]
            sidx_cols.append(_wrap16(stream.astype(np.int16), 8 * kj, zp))
        sidx16 = np.concatenate(sidx_cols, axis=1)

        orders.append(order)
        in_maps.append({
            "feat": feat,
            "gidx": np.ascontiguousarray(gidx16),
            "sidx": np.ascontiguousarray(sidx16),
            "recip": np.ascontiguousarray(recip[c * b_loc:(c + 1) * b_loc][order]),
        })
    return in_maps, orders, k_sched


def kernel(features, neigh_idx, neigh_counts):
    from concourse.bass_utils import run_bass_kernel_spmd

    in_maps, orders, k_sched = prep_core_inputs(
        features, neigh_idx, neigh_counts)
    cw_sched = _LAST_SCHED
    key = ("nc", N_NODES, BATCH // N_CORES, MAX_NEIGH, FEAT_DIM, k_sched)
    if key not in _KERNEL_CACHE:
        _KERNEL_CACHE[key] = build_nc(k_sched=list(k_sched),
                                      cw_sched=list(cw_sched))
    nc = _KERNEL_CACHE[key]

    res = run_bass_kernel_spmd(nc, in_maps, list(range(N_CORES)))
    b_loc = BATCH // N_CORES
    out = np.empty((BATCH, FEAT_DIM), dtype=np.float32)
    for c in range(N_CORES):
        out_c = np.empty((b_loc, FEAT_DIM), dtype=np.float32)
        out_c[orders[c]] = res.results[c]["out"]
        out[c * b_loc:(c + 1) * b_loc] = out_c
    return out
